# revision 1
# baseline (speedup 1.0000x reference)
"""ConvLogicTree layer for Trainium2 (8 NeuronCores, SPMD data-parallel over batch).

Math: the 16 soft binary gates are all affine in the monomial basis
[1, a, b, a*b], so softmax-gate-mixing per tree node collapses to
    node(a, b) = k0 + ka*a + kb*b + kab*(a*b)
with per-(channel, node) coefficients k = softmax(w) @ C  (C = gate->monomial
matrix).  Each output channel's 8 leaves are shifted 3x3-window views of 2
input channels; we materialize a 9-shift unfold U in DRAM scratch and pull the
1024 needed (shift, channel) rows per core with a single indexed dma_gather.
"""

import os
import sys

sys.path.insert(0, "/opt/trn_rl_repo")

import numpy as np

import concourse.bass as bass
import concourse.bacc as bacc
import concourse.mybir as mybir
import concourse.tile as tile
from contextlib import ExitStack
from concourse.bass_utils import run_bass_kernel_spmd
from concourse.library_config import mlp

F32 = mybir.dt.float32
I16 = mybir.dt.int16
AF = mybir.ActivationFunctionType
ALU = mybir.AluOpType

N_CORES = 8
B, C_IN, H, W = 16, 64, 32, 32
C_OUT = 128
NB = B // N_CORES          # batches per core
L = H * W                  # 1024 pixels
FD = NB * L                # free dim per compute op (batch-major pixels)
N_LEAF = 8

# gate g -> coefficients on [1, a, b, ab]
GATE_C = np.array(
    [
        [0, 0, 0, 0],    # 0
        [0, 0, 0, 1],    # ab
        [0, 1, 0, -1],   # a - ab
        [0, 1, 0, 0],    # a
        [0, 0, 1, -1],   # b - ab
        [0, 0, 1, 0],    # b
        [0, 1, 1, -2],   # a + b - 2ab
        [0, 1, 1, -1],   # a + b - ab
        [1, -1, -1, 1],  # 1 - (a+b-ab)
        [1, -1, -1, 2],  # 1 - (a+b-2ab)
        [1, 0, -1, 0],   # 1 - b
        [1, 0, -1, 1],   # 1 - b + ab
        [1, -1, 0, 0],   # 1 - a
        [1, -1, 0, 1],   # 1 - a + ab
        [1, 0, 0, -1],   # 1 - ab
        [1, 0, 0, 0],    # 1
    ],
    dtype=np.float32,
)

# tree wiring: (level, pair) -> weight row;  rows overlap across levels
# (faithful to the module: gate_idx = 2**level - 1 + pair)
L0_ROWS = [0, 1, 2, 3]
L1_ROWS = [1, 2]
L2_ROW = 3


def build_program():
    nc = bacc.Bacc("TRN2", target_bir_lowering=False, debug=False)

    x_in = nc.dram_tensor("x", [128, L], F32, kind="ExternalInput")
    w_in = nc.dram_tensor("w", [C_OUT, 7, 16], F32, kind="ExternalInput")
    cm_in = nc.dram_tensor("cmat", [128, 4, 7, 16], F32, kind="ExternalInput")
    gi_in = nc.dram_tensor("gidx", [128, 128], I16, kind="ExternalInput")
    out_ext = nc.dram_tensor("out", [NB, C_OUT, L], F32, kind="ExternalOutput")
    # 9-shift unfold scratch: row (s*128 + b*64 + c) holds shift-s of channel
    # c, batch b — full-128-partition writes straight from the xpad tile
    u_dram = nc.dram_tensor("u", [9 * NB * C_IN, L], F32)

    with tile.TileContext(nc) as tc, ExitStack() as ctx:
        pool = ctx.enter_context(tc.tile_pool(name="p", bufs=1))
        tmp = ctx.enter_context(tc.tile_pool(name="tmp", bufs=2))

        wt = pool.tile([128, 7, 16], F32)
        cm = pool.tile([128, 4, 7, 16], F32)
        en = pool.tile([128, 7, 16], F32)
        ssum = pool.tile([128, 7], F32)
        srec = pool.tile([128, 7], F32)
        km = pool.tile([128, 4, 7], F32)
        xp = pool.tile([128, 34 * 34], F32)
        gidx = pool.tile([128, 128], I16)
        lv = pool.tile([128, N_LEAF, FD], F32)
        nodes = [pool.tile([128, FD], F32, name=f"n{i}", tag=f"n{i}") for i in range(4)]
        mids = [pool.tile([128, FD], F32, name=f"m{i}", tag=f"m{i}") for i in range(2)]
        ot = pool.tile([128, FD], F32)

        nc.sync.dma_start(out=wt[:], in_=w_in[:])
        nc.sync.dma_start(out=cm[:], in_=cm_in[:])
        nc.sync.dma_start(out=gidx[:], in_=gi_in[:])

        # ---- softmax(w) @ C -> km[m, row]   (no max-subtraction: |w| ~ N(0,1))
        nc.scalar.activation(en[:], wt[:], AF.Exp)
        nc.vector.tensor_reduce(ssum[:], en[:], axis=mybir.AxisListType.X, op=ALU.add)
        nc.vector.reciprocal(srec[:], ssum[:])
        for n in range(7):
            nc.vector.tensor_scalar(
                en[:, n], en[:, n], srec[:, n : n + 1], None, op0=ALU.mult
            )
        for m in range(4):
            prd = tmp.tile([128, 7, 16], F32, tag="prd")
            nc.vector.tensor_tensor(prd[:], en[:], cm[:, m], op=ALU.mult)
            nc.vector.tensor_reduce(
                km[:, m], prd[:], axis=mybir.AxisListType.X, op=ALU.add
            )

        # ---- zero-padded input image per (b, c) partition
        nc.vector.memset(xp[:], 0.0)
        xpv = xp[:].rearrange("p (r c) -> p r c", r=34)
        nc.sync.dma_start(
            out=xpv[:, 1:33, 1:33],
            in_=x_in[:].rearrange("p (r c) -> p r c", r=32),
        )

        # ---- 9-shift unfold written to DRAM scratch; alternate between the
        # two HWDGE rings (SP / ACT) so the writes drain in parallel
        for s in range(9):
            ki, kj = s // 3, s % 3
            src = xpv[:, ki : ki + 32, kj : kj + 32]
            dst = u_dram[s * 128 : (s + 1) * 128, :]
            eng = nc.sync if s % 2 == 0 else nc.scalar
            eng.dma_start(out=dst, in_=src)

        # ---- derived coefficients for the factored node form
        #   node(a,b) = kab*(a + alpha)*(b + beta) + delta
        #   alpha = kb/kab, beta = ka/kab, delta = k0 - ka*kb/kab
        # (numerically safe here: |ka*kb/kab| stays tiny for softmax blends,
        #  verified against the host-side error proxy)
        alp = pool.tile([128, 7], F32)
        bet = pool.tile([128, 7], F32)
        dlt = pool.tile([128, 7], F32)
        rkab = pool.tile([128, 7], F32)
        nc.vector.reciprocal(rkab[:], km[:, 3])
        nc.vector.tensor_tensor(alp[:], km[:, 2], rkab[:], op=ALU.mult)
        nc.vector.tensor_tensor(bet[:], km[:, 1], rkab[:], op=ALU.mult)
        nc.vector.tensor_tensor(dlt[:], alp[:], km[:, 1], op=ALU.mult)
        nc.vector.tensor_tensor(dlt[:], km[:, 0], dlt[:], op=ALU.subtract)

        # ---- gather the 8 leaves, one call per level-0 pair (overlaps compute)
        nc.gpsimd.load_library(mlp)
        for p in range(4):
            nc.gpsimd.dma_gather(
                lv[:, 2 * p : 2 * p + 2].rearrange("p j (b f) -> p (j b) f", b=NB),
                u_dram[:],
                gidx[:, p * 32 : (p + 1) * 32],
                512,
                512,
                L,
            )

        # ---- tree:  node(a,b) = kab*(a+alpha)*(b+beta) + delta
        # engine split per node: Pool computes (b+beta), DVE the product via
        # scalar_tensor_tensor, ACT the final scale+shift (in-place).
        def emit_node(a_ap, b_ap, row, out_tile, t_eng):
            a_col = alp[:, row : row + 1]
            b_col = bet[:, row : row + 1]
            d_col = dlt[:, row : row + 1]
            kab = km[:, 3, row : row + 1]
            t = tmp.tile([128, FD], F32, tag="t")
            if t_eng == "dve":
                nc.vector.tensor_scalar(t[:], b_ap, b_col, None, op0=ALU.add)
            elif t_eng == "act":
                nc.scalar.activation(t[:], b_ap, AF.Identity, bias=b_col, scale=1.0)
            else:
                nc.gpsimd.tensor_scalar(t[:], b_ap, b_col, None, op0=ALU.add)
            nc.vector.scalar_tensor_tensor(
                out_tile[:], a_ap, a_col, t[:], op0=ALU.add, op1=ALU.mult
            )
            nc.scalar.activation(
                out_tile[:], out_tile[:], AF.Identity, bias=d_col, scale=kab
            )

        l0_eng = ["dve", "act", "dve", "act"]
        for p in range(4):
            emit_node(lv[:, 2 * p], lv[:, 2 * p + 1], L0_ROWS[p], nodes[p], l0_eng[p])
        emit_node(nodes[0][:], nodes[1][:], L1_ROWS[0], mids[0], "pool")
        emit_node(nodes[2][:], nodes[3][:], L1_ROWS[1], mids[1], "pool")
        emit_node(mids[0][:], mids[1][:], L2_ROW, ot, "pool")

        nc.sync.dma_start(
            out=out_ext[:].rearrange("b o f -> o b f"),
            in_=ot[:].rearrange("p (b f) -> p b f", b=NB),
        )

    nc.compile()
    return nc


def make_host_inputs(x, weights, leaf_indices):
    """Shared input prep: per-core in_maps (kernel shards batch over cores)."""
    x = np.ascontiguousarray(np.asarray(x), dtype=np.float32)
    weights = np.ascontiguousarray(np.asarray(weights), dtype=np.float32)
    leaf_indices = np.asarray(leaf_indices)

    feat = leaf_indices.astype(np.int64)          # [C_OUT, 8]
    c = feat // 9
    tap = feat % 9
    # U row = s*128 + b*64 + c ; gather order i = (j*NB + b)*128 + o
    order = np.zeros(2048, np.int16)
    for j in range(8):
        for b in range(NB):
            blk = j * NB + b
            order[blk * 128 : (blk + 1) * 128] = (
                tap[:, j] * 128 + b * C_IN + c[:, j]
            ).astype(np.int16)
    wrapped = np.zeros((16, 128), np.int16)
    ii = np.arange(2048)
    wrapped[ii % 16, ii // 16] = order[ii]
    gidx = np.tile(wrapped, (8, 1))               # replicated per Q7 core

    cmat = np.ascontiguousarray(
        np.broadcast_to(GATE_C.T.reshape(1, 4, 1, 16), (128, 4, 7, 16)),
        dtype=np.float32,
    )

    in_maps = []
    for core in range(N_CORES):
        xs = np.ascontiguousarray(
            x[core * NB : (core + 1) * NB].reshape(128, L)
        )
        in_maps.append({"x": xs, "w": weights, "cmat": cmat, "gidx": gidx})
    return in_maps


_NC_CACHE = {}


def kernel(x, weights, leaf_indices):
    key = "prog"
    if key not in _NC_CACHE:
        _NC_CACHE[key] = build_program()
    nc = _NC_CACHE[key]
    in_maps = make_host_inputs(x, weights, leaf_indices)
    res = run_bass_kernel_spmd(nc, in_maps, list(range(N_CORES)))
    out = np.concatenate(
        [r["out"].reshape(NB, C_OUT, H, W) for r in res.results], axis=0
    )
    return out



# revision 7
# speedup vs baseline: 1.7246x; 1.7246x over previous
"""ConvLogicTree layer for Trainium2 (8 NeuronCores, SPMD data-parallel over batch).

Math: the 16 soft binary gates are all affine in the monomial basis
[1, a, b, a*b], so softmax-gate-mixing per tree node collapses to
    node(a, b) = kab*(a + alpha)*(b + beta) + delta
with per-(channel, node) coefficients k = softmax(w) @ C.  All coefficient
algebra (softmax, the factored form, folding each node's delta into the
next level's affine) is done host-side in f64; the device kernel is:

  1. load x [128=(b,c), 1024] f32, cast to bf16
  2. DVE shift-copies build the 9-tap zero-padded unfold U9 in SBUF (bf16)
  3. one contiguous DMA writes U9 to DRAM as rows (tap*64+c) of [2*1024]
  4. gpsimd dma_gather pulls the 8 leaf rows per output channel (4 calls,
     one per level-0 pair, so tree compute overlaps the gather)
  5. the 7-node tree runs on DVE (tensor_scalar/tensor_tensor) + ACT
     (activation scale+bias), nothing on gpsimd
  6. one DMA writes the f32 output

Leaves are bf16 (halves gather bytes); every affine uses the scale+bias
form (coeffs O(1), no huge intermediates), products run in f32.
"""

import sys

sys.path.insert(0, "/opt/trn_rl_repo")

import numpy as np

import concourse.bass as bass
import concourse.bacc as bacc
import concourse.mybir as mybir
import concourse.tile as tile
from contextlib import ExitStack
from concourse.bass_utils import run_bass_kernel_spmd
from concourse.library_config import mlp

F32 = mybir.dt.float32
BF16 = mybir.dt.bfloat16
I16 = mybir.dt.int16
AF = mybir.ActivationFunctionType
ALU = mybir.AluOpType

N_CORES = 8
B, C_IN, H, W = 16, 64, 32, 32
C_OUT = 128
NB = B // N_CORES          # batches per core
L = H * W                  # 1024 pixels
FD = NB * L                # free dim per compute op (batch-major pixels)

# gate g -> coefficients on [1, a, b, ab]
GATE_C = np.array(
    [
        [0, 0, 0, 0],    # 0
        [0, 0, 0, 1],    # ab
        [0, 1, 0, -1],   # a - ab
        [0, 1, 0, 0],    # a
        [0, 0, 1, -1],   # b - ab
        [0, 0, 1, 0],    # b
        [0, 1, 1, -2],   # a + b - 2ab
        [0, 1, 1, -1],   # a + b - ab
        [1, -1, -1, 1],  # 1 - (a+b-ab)
        [1, -1, -1, 2],  # 1 - (a+b-2ab)
        [1, 0, -1, 0],   # 1 - b
        [1, 0, -1, 1],   # 1 - b + ab
        [1, -1, 0, 0],   # 1 - a
        [1, -1, 0, 1],   # 1 - a + ab
        [1, 0, 0, -1],   # 1 - ab
        [1, 0, 0, 0],    # 1
    ],
    dtype=np.float64,
)

# tree wiring: (level, pair) -> weight row; rows overlap across levels
# (faithful to the module: gate_idx = 2**level - 1 + pair)
L0_ROWS = [0, 1, 2, 3]
L1_ROWS = [1, 2]
L2_ROW = 3

# scalar-tile column layout: per output channel o, [128, 22] f32
#  0..3   L0 A-scale   = kab0_p           (A = a*kab0 + kb0)
#  4..7   L0 A-bias    = kb0_p            (= kab0*alpha0)
#  8..11  L0 B-bias    = beta0_p          (Bt = b + beta0)
# 12..13  L1 X-scale   = kab1_q
# 14..15  L1 X-bias    = kab1_q*(delta0_{2q} + alpha1_q)
# 16..17  L1 Y-bias    = delta0_{2q+1} + beta1_q
# 18      R X-scale    = kabR
# 19      R X-bias     = kabR*(delta1_0 + alphaR)
# 20      R Y-bias     = delta1_1 + betaR
# 21      R out-bias   = deltaR
N_SC = 22


def build_program():
    nc = bacc.Bacc("TRN2", target_bir_lowering=False, debug=False)

    x_in = nc.dram_tensor("x", [128, L], F32, kind="ExternalInput")
    sc_in = nc.dram_tensor("sc", [128, N_SC], F32, kind="ExternalInput")
    gi_in = nc.dram_tensor("gidx", [128, 64], I16, kind="ExternalInput")
    out_ext = nc.dram_tensor("out", [NB, C_OUT, L], F32, kind="ExternalOutput")
    # unfold scratch: row (tap*64 + c) holds [b0 pixels | b1 pixels] bf16
    u_dram = nc.dram_tensor("u", [9 * C_IN, NB * L], BF16)

    with tile.TileContext(nc) as tc, ExitStack() as ctx:
        pool = ctx.enter_context(tc.tile_pool(name="p", bufs=1))
        tmp = ctx.enter_context(tc.tile_pool(name="tmp", bufs=2))

        xt = pool.tile([128, L], F32)
        xb = pool.tile([128, 32, 32], BF16)
        u9 = pool.tile([128, 9, L], BF16)
        sc = pool.tile([128, N_SC], F32)
        gidx = pool.tile([128, 64], I16)
        lv = pool.tile([128, 8, FD], BF16)
        pt = [pool.tile([128, FD], F32, name=f"P{i}", tag=f"P{i}") for i in range(4)]
        mt = [pool.tile([128, FD], F32, name=f"M{i}", tag=f"M{i}") for i in range(2)]
        ot = pool.tile([128, FD], F32)

        nc.gpsimd.load_library(mlp)
        nc.scalar.dma_start(out=sc[:], in_=sc_in[:])
        nc.scalar.dma_start(out=gidx[:], in_=gi_in[:])
        nc.sync.dma_start(out=xt[:], in_=x_in[:])

        # border zeros for the 8 off-center taps live only in u9
        nc.vector.memset(u9[:], 0.0)
        nc.vector.tensor_scalar(xb[:], xt[:].rearrange("p (r c) -> p r c", r=32),
                                0.0, None, op0=ALU.add)

        # 9 shifted zero-padded copies; tap s=(ki,kj) reads x rows r+ki-1
        xv = xb[:]
        for s in range(9):
            ki, kj = s // 3, s % 3
            dy, dx = ki - 1, kj - 1
            r0, r1 = max(0, -dy), 32 - max(0, dy)
            c0, c1 = max(0, -dx), 32 - max(0, dx)
            dst = u9[:, s].rearrange("p (r c) -> p r c", r=32)[:, r0:r1, c0:c1]
            src = xv[:, r0 + dy:r1 + dy, c0 + dx:c1 + dx]
            eng = nc.vector if s % 3 != 2 else nc.scalar
            if eng is nc.vector:
                eng.tensor_scalar(dst, src, 0.0, None, op0=ALU.add)
            else:
                eng.activation(dst, src, AF.Identity, bias=0.0, scale=1.0)
            # write this tap's 64 rows as soon as they're ready
            dring = nc.sync if s % 2 == 0 else nc.scalar
            dst_dram = u_dram[:].rearrange(
                "(s c) (b f) -> s b c f", s=9, b=NB
            )[s]
            dring.dma_start(out=dst_dram, in_=u9[:, s])

        # ---- gather the 8 leaves, one call per level-0 pair
        for p in range(4):
            nc.gpsimd.dma_gather(
                lv[:, 2 * p:2 * p + 2],
                u_dram[:],
                gidx[:, p * 16:(p + 1) * 16],
                256,
                256,
                FD,
            )

        # ---- tree; every affine is scale+bias with O(1) coefficients
        def col(i):
            return sc[:, i:i + 1]

        for p in range(4):
            a_ap, b_ap = lv[:, 2 * p], lv[:, 2 * p + 1]
            at = tmp.tile([128, FD], F32, tag="a")
            bt = tmp.tile([128, FD], F32, tag="b")
            # A = a*kab0 + kb0 on DVE; Bt = b + beta0 on ACT (parallel)
            nc.vector.tensor_scalar(at[:], a_ap, col(p), col(4 + p),
                                    op0=ALU.mult, op1=ALU.add)
            nc.scalar.activation(bt[:], b_ap, AF.Identity, bias=col(8 + p),
                                 scale=1.0)
            nc.vector.tensor_tensor(pt[p][:], at[:], bt[:], op=ALU.mult)

        for q in range(2):
            xq = tmp.tile([128, FD], F32, tag="x")
            yq = tmp.tile([128, FD], F32, tag="y")
            nc.vector.tensor_scalar(xq[:], pt[2 * q][:], col(12 + q),
                                    col(14 + q), op0=ALU.mult, op1=ALU.add)
            nc.scalar.activation(yq[:], pt[2 * q + 1][:], AF.Identity,
                                 bias=col(16 + q), scale=1.0)
            nc.vector.tensor_tensor(mt[q][:], xq[:], yq[:], op=ALU.mult)

        xr = tmp.tile([128, FD], F32, tag="x")
        yr = tmp.tile([128, FD], F32, tag="y")
        nc.vector.tensor_scalar(xr[:], mt[0][:], col(18), col(19),
                                op0=ALU.mult, op1=ALU.add)
        nc.vector.tensor_scalar(yr[:], mt[1][:], col(20), None, op0=ALU.add)
        nc.vector.tensor_tensor(ot[:], xr[:], yr[:], op=ALU.mult)
        nc.vector.tensor_scalar(ot[:], ot[:], col(21), None, op0=ALU.add)

        nc.sync.dma_start(
            out=out_ext[:].rearrange("b o f -> o b f"),
            in_=ot[:].rearrange("p (b f) -> p b f", b=NB),
        )

    nc.compile()
    return nc


def _softmax64(w):
    e = np.exp(w - w.max(axis=-1, keepdims=True))
    return e / e.sum(axis=-1, keepdims=True)


def make_host_inputs(x, weights, leaf_indices):
    """Shared input prep: per-core in_maps (kernel shards batch over cores)."""
    x = np.ascontiguousarray(np.asarray(x), dtype=np.float32)
    weights = np.asarray(weights, dtype=np.float64)
    leaf_indices = np.asarray(leaf_indices).astype(np.int64)  # [C_OUT, 8]

    # ---- gather indices: u row = tap*64 + c; gather idx i = j*128 + o
    c = leaf_indices // 9
    tap = leaf_indices % 9
    order = (tap.T * C_IN + c.T).reshape(-1).astype(np.int16)  # [8*128], j-major
    wrapped = np.zeros((16, 64), np.int16)
    ii = np.arange(1024)
    wrapped[ii % 16, ii // 16] = order
    gidx = np.tile(wrapped, (8, 1))  # [128, 64], replicated per Q7 core

    # ---- per-node factored coefficients in f64
    km = _softmax64(weights) @ GATE_C  # [128, 7, 4] -> k0, ka, kb, kab
    def coef(row):
        k0, ka, kb, kab = (km[:, row, i] for i in range(4))
        return kb / kab, ka / kab, k0 - ka * kb / kab, kab  # alpha, beta, delta

    a0, b0, d0, kab0 = zip(*[coef(r) for r in L0_ROWS])
    a1, b1, d1, kab1 = zip(*[coef(r) for r in L1_ROWS])
    aR, bR, dR, kabR = coef(L2_ROW)

    sc = np.zeros((128, N_SC), np.float64)
    for p in range(4):
        sc[:, p] = kab0[p]
        sc[:, 4 + p] = kab0[p] * a0[p]
        sc[:, 8 + p] = b0[p]
    for q in range(2):
        sc[:, 12 + q] = kab1[q]
        sc[:, 14 + q] = kab1[q] * (d0[2 * q] + a1[q])
        sc[:, 16 + q] = d0[2 * q + 1] + b1[q]
    sc[:, 18] = kabR
    sc[:, 19] = kabR * (d1[0] + aR)
    sc[:, 20] = d1[1] + bR
    sc[:, 21] = dR
    sc = np.ascontiguousarray(sc, dtype=np.float32)

    in_maps = []
    for core in range(N_CORES):
        xs = np.ascontiguousarray(
            x[core * NB:(core + 1) * NB].reshape(128, L)
        )
        in_maps.append({"x": xs, "sc": sc, "gidx": gidx})
    return in_maps


_NC_CACHE = {}


def kernel(x, weights, leaf_indices):
    key = "prog"
    if key not in _NC_CACHE:
        _NC_CACHE[key] = build_program()
    nc = _NC_CACHE[key]
    in_maps = make_host_inputs(x, weights, leaf_indices)
    res = run_bass_kernel_spmd(nc, in_maps, list(range(N_CORES)))
    out = np.concatenate(
        [r["out"].reshape(NB, C_OUT, H, W) for r in res.results], axis=0
    )
    return out
